# revision 33
# baseline (speedup 1.0000x reference)
"""Multi-head attention (B=4, S=2048, D=1024, H=16, hd=64) on 8 NeuronCores.

Tensor-parallel over heads: core c computes heads 2c, 2c+1, producing a
partial output that the host sums across cores (row-sharded Wo).

v4 (296.4us, from the 310.2us v2 baseline):
- QKV projections in fp8e4m3 DoubleRow perf mode (two contraction rows
  per partition, 0.5 PE cycles/row). Precision recovered with an
  error-compensated 3-pass split: W and x decomposed on the host into
  fp8 hi + fp8 residual; the projection accumulates W_hi*x_hi + W_hi*x_lo
  + W_lo*x_hi in fp32 PSUM (dropped lo*lo term ~1e-5 relative). W is
  pre-scaled by 32 into e4m3's normal range; the inverse scale folds into
  the exp() activation scale (scores) and host-side Wo scaling (V path).
  QKV is the only contraction deep enough for the split to win; scores/
  AV/out-proj have no accumulation depth to amortize it and stay fp16.
- The ACT engine is the bottleneck (256 exps x 1038ns = 266us busy);
  the schedule keeps its exp stream dense: score groups emit in PAIRS
  (the 2-buffer PSUM score pool allows two groups in flight) so slot
  jitter tolerance doubles; per-pair PE drain work is quota-spread and
  hard-capped in the last batch.
- x ships as fp8 hi+lo halves (same bytes as fp16); the hi half-chunk
  DMA alone unblocks 8 of the 12 fill matmuls. Output DMA is fp16
  (host accumulates in fp32). PE p-state warmup matmuls burn the
  half-clock ramp before the first DMAs land.
- ps_o pool round-robin safety: fill chains hold a PSUM bank across two
  units; a rotation-parity tracker blocks any allocation that would
  land on a bank with an open accumulator.
- Post-exp drain: normalize/copies split across ACT (idle by then) and
  DVE; out-proj consumes a PE-transposed fp16 context.

v2 retained: attn@V flipped - at2 (exp scores, [keys, q]) stationary,
V ([keys, 65] fp16 with a ones column accumulating Z) moving. Softmax
normalization is a per-partition scalar multiply. QKV of batch b+1 and
the attention tail of the previous q-chunk interleave into the score
phase.
"""

import sys

sys.path.insert(0, "/opt/trn_rl_repo")

import numpy as np
import ml_dtypes
import concourse.bass as bass
import concourse.bacc as bacc
import concourse.mybir as mybir
import concourse.tile as tile
from concourse.bass_utils import run_bass_kernel_spmd

F32 = mybir.dt.float32
F32R = mybir.dt.float32r
F16 = mybir.dt.float16
F8 = mybir.dt.float8e4
DR = mybir.MatmulPerfMode.DoubleRow
AF = mybir.ActivationFunctionType
E4M3 = ml_dtypes.float8_e4m3

B, S, D = 4, 2048, 1024
SEQ = B * S
NCORES = 8
DPC = 128            # dims per core = 2 heads * 64
KT = D // 128        # 8 k-tiles for the QKV contraction
F = 512              # free-dim chunk
NSC = S // F         # seq chunks per batch = 4
NKB = S // 128       # key blocks per batch = 16
NQB = S // 128       # q blocks per batch = 16
WSC = 32.0           # host-side scale on W before fp8 quantization
# scores PSUM = (32*Q)·(32*K); exp scale folds 1/(32*32) and 1/sqrt(hd)
EXP_SCALE = 1.0 / (WSC * WSC * 8.0)
# fp8 3-pass: (w half, x half) per pass; hi=0, lo=1
PASSES = ((0, 0), (0, 1), (1, 0))

_CACHE = {}


def _build():
    nc = bacc.Bacc("TRN2", target_bir_lowering=False, debug=False,
                   enable_asserts=False)

    # x and the QKV weights travel as fp8 hi+lo pairs (same bytes as fp16)
    xT_d = nc.dram_tensor("xT8", [128, 2 * KT * SEQ], F8,
                          kind="ExternalInput")
    wq_d = nc.dram_tensor("wq8", [128, 2 * KT * DPC], F8,
                          kind="ExternalInput")
    wk_d = nc.dram_tensor("wk8", [128, 2 * KT * DPC], F8,
                          kind="ExternalInput")
    wv_d = nc.dram_tensor("wv8", [128, 2 * KT * DPC], F8,
                          kind="ExternalInput")
    # identh (128) + woT (1024) merged: one DMA instead of three
    const_d = nc.dram_tensor("constT", [128, 128 + D], F16,
                             kind="ExternalInput")
    bqk_d = nc.dram_tensor("bqk", [DPC, 2], F32, kind="ExternalInput")
    out_d = nc.dram_tensor("out", [SEQ, D], F16, kind="ExternalOutput")

    with tile.TileContext(nc) as tc:
        with (
            tc.tile_pool(name="wp", bufs=1) as wp,
            tc.tile_pool(name="xp", bufs=3) as xp,
            tc.tile_pool(name="qk", bufs=2) as qk,
            tc.tile_pool(name="vp", bufs=2) as vp,
            tc.tile_pool(name="ap", bufs=32) as apool,
            tc.tile_pool(name="nx", bufs=8) as nxp,
            tc.tile_pool(name="cx", bufs=64) as cxp,
            tc.tile_pool(name="zp", bufs=8) as zp,
            tc.tile_pool(name="op", bufs=4) as op,
            # PSUM bank budget (8 total): sc 2x2 + ct 2x1 + shared misc 2x1
            tc.tile_pool(name="ps_sc", bufs=2, space=bass.MemorySpace.PSUM) as psb,
            tc.tile_pool(name="ps_ct", bufs=2, space=bass.MemorySpace.PSUM) as psc,
            tc.tile_pool(name="ps_o", bufs=2, space=bass.MemorySpace.PSUM) as pso,
        ):
            # resident weights / constants ([p, hi/lo, kt, m] fp8)
            wq_sb = wp.tile([128, KT, 2, DPC], F8, tag="wq")
            wk_sb = wp.tile([128, KT, 2, DPC], F8, tag="wk")
            wv_sb = wp.tile([128, KT, 2, DPC], F8, tag="wv")
            const_sb = wp.tile([128, 128 + D], F16, tag="cn")
            identh = const_sb[:, 0:128]
            wo_sb = const_sb[:, 128:128 + D]
            bqk_sb = wp.tile([DPC, 2], F32, tag="bq")
            bq_sb = bqk_sb[:, 0:1]
            bk_sb = bqk_sb[:, 1:2]
            xT_r = xT_d[:].rearrange("p (hl kt f) -> p hl kt f", hl=2, kt=KT)

            def load_fill_weights():
                # critical boot path: biases then K/V weights
                nc.sync.dma_start(bqk_sb[:], bqk_d[:])
                nc.sync.dma_start(
                    wk_sb[:].rearrange("p kt hl m -> p (kt hl m)"), wk_d[:])
                nc.sync.dma_start(
                    wv_sb[:].rearrange("p kt hl m -> p (kt hl m)"), wv_d[:])

            def load_const():
                # identh first needed by u_ctxT (~slot 16), woT by u_oproj
                nc.sync.dma_start(const_sb[:], const_d[:])

            st = [dict() for _ in range(B)]   # per-batch tiles
            drain_mode = {"on": False}  # True once every exp is emitted

            # ps_o pool rotation state: `cnt` counts every allocation from
            # the pool (slot = cnt % 2); `live` maps open fill chains to the
            # slot their accumulator holds. A unit may only allocate if its
            # slot(s) are not held by a live chain, else the round-robin
            # rotation would hand out a bank whose accumulator still has
            # unemitted matmuls (silent clobber).
            psoT = {"cnt": 0, "live": {}}

            def pso_ok(kind):
                if kind in ("", "mid", "close"):
                    return True
                if kind == "pso2":     # allocates two tiles (u_oproj)
                    return not psoT["live"]
                return (psoT["cnt"] % 2) not in psoT["live"].values()

            # ---- phase-A unit builders (QKV projection for batch b) --------
            def a_units(b, defer_boot=False):
                units = []
                dma_units = []
                fill_units = []
                by_kind = []    # (q_units, k_units, v_units) per sc

                def u_start():
                    st[b]["qt"] = qk.tile([128, S], F32R, tag="qt",
                                          name=f"qt{b}")
                    st[b]["kt"] = qk.tile([128, S], F32R, tag="kt",
                                          name=f"kt{b}")
                    # per key block: [h0 d(64) | ones | h1 d(64) | ones]
                    va = vp.tile([128, NKB, 130], F16, tag="va",
                                 name=f"va{b}")
                    st[b]["va"] = va
                    nc.vector.memset(va[:, :, 64:65], 1.0)
                    nc.vector.memset(va[:, :, 129:130], 1.0)
                units.append((0, u_start, ""))

                for sc in range(NSC):
                    def u_dma(sc=sc):
                        # two half-chunk DMAs: descriptor generation costs
                        # ~625ns per dma_start, so batch k-tiles together
                        xt = xp.tile([128, 2, KT, F], F8, tag="xt",
                                     name=f"xt{b}_{sc}")
                        st[b][f"xt{sc}"] = xt
                        lo = b * S + sc * F
                        hk = KT // 2
                        nc.sync.dma_start(xt[:, 0, :, :],
                                          xT_r[:, 0, :, lo:lo + F])
                        nc.sync.dma_start(xt[:, 1, :, :],
                                          xT_r[:, 1, :, lo:lo + F])
                    dma_units.append((0, u_dma, ""))

                    # q/k fill: 12 DoubleRow matmuls (3 fp8 passes x 4
                    # k-pairs) accumulating into one PSUM tile, emitted as
                    # two 6-matmul units (same 2-unit chain shape as the
                    # tuned fp16 schedule so the pso pool rotation pattern
                    # is unchanged)
                    def u_fill(sc, half, w_sb, b_sb, dst_kind):
                        xt = st[b][f"xt{sc}"]
                        key = (b, sc, dst_kind)
                        if half == 0:
                            ps = pso.tile([128, F], F32, tag="o",
                                          name=f"ps{b}_{sc}_{dst_kind}")
                            st[b]["fillps"] = ps
                            psoT["live"][key] = psoT["cnt"] % 2
                            psoT["cnt"] += 1
                        else:
                            ps = st[b]["fillps"]
                            del psoT["live"][key]
                        # steps grouped by x half: unit 0 (8 matmuls)
                        # only needs the hi half-chunk DMA; unit 1 (4)
                        # consumes the lo half
                        steps = [(PASSES[pi], j)
                                 for pi in (0, 2, 1)
                                 for j in range(KT // 2)]
                        rng = range(0, 8) if half == 0 else range(8, 12)
                        for n in rng:
                            (hw, hx), j = steps[n]
                            nc.tensor.matmul(ps[:],
                                             w_sb[:, 2 * j:2 * j + 2, hw, :],
                                             xt[:, hx, 2 * j:2 * j + 2, :],
                                             start=(n == 0), stop=(n == 11),
                                             perf_mode=DR)
                        if half == 1:
                            dst = st[b][dst_kind]
                            nc.vector.tensor_scalar_add(
                                dst[:, sc * F:(sc + 1) * F], ps[:],
                                b_sb[:])

                    # v fill: per unit, one 128-key half of the chunk as
                    # 24 DoubleRow matmuls (2 blocks x 3 passes x 4
                    # k-pairs); V lands transposed [keys, dims] with xt
                    # stationary
                    def u_vfill(sc, half):
                        xt = st[b][f"xt{sc}"]
                        va = st[b]["va"]
                        ps = pso.tile([128, F], F32, tag="o",
                                      name=f"vp{b}_{sc}_{half}")
                        psoT["cnt"] += 1
                        psv = ps[:, 0:256].rearrange("p (bl m) -> p bl m",
                                                     bl=2)
                        for blk in range(2):
                            i = half * 2 + blk
                            for n, ((hw, hx), j) in enumerate(
                                    (PASSES[pi], j)
                                    for pi in range(3)
                                    for j in range(KT // 2)):
                                nc.tensor.matmul(
                                    psv[:, blk, :],
                                    xt[:, hx, 2 * j:2 * j + 2,
                                       i * 128:(i + 1) * 128],
                                    wv_sb[:, 2 * j:2 * j + 2, hw, :],
                                    start=(n == 0), stop=(n == 11),
                                    perf_mode=DR)
                        kb = sc * (F // 128) + half * 2
                        for blk in range(2):
                            dstv = va[:, kb + blk, 0:130].rearrange(
                                "p (g x) -> p g x", g=2)[:, :, 0:64]
                            srcv = psv[:, blk, :].rearrange(
                                "p (g x) -> p g x", g=2)
                            nc.vector.tensor_copy(dstv, srcv)

                    HK = ("open", "close")
                    PEH = (853, 427)
                    qu = [(PEH[h], (lambda sc=sc, h=h: u_fill(sc, h, wq_sb,
                                                              bq_sb, "qt")),
                           HK[h]) for h in range(2)]
                    ku = [(PEH[h], (lambda sc=sc, h=h: u_fill(sc, h, wk_sb,
                                                              bk_sb, "kt")),
                           HK[h]) for h in range(2)]
                    vu = [(640, (lambda sc=sc, h=h: u_vfill(sc, h)), "pso")
                          for h in range(2)]
                    fu = qu + ku + vu
                    fill_units.append(fu)
                    by_kind.append((qu, ku, vu))
                # prefetch xt one chunk ahead of the fills that consume it
                units.append(dma_units[0])
                for sc in range(NSC):
                    if sc + 1 < NSC:
                        units.append(dma_units[sc + 1])
                    units.extend(fill_units[sc])
                if not defer_boot:
                    return units
                # boot: emit only chunk 0 up front so the first q-chunk's
                # score phase can start; defer the rest into its slots.
                # k-fills lead (score units consume key chunks in order),
                # and ALL va fills must drain before qc0's AV units.
                boot_units = ([units[0], dma_units[0], dma_units[1]]
                              + fill_units[0])
                # k-fills FIRST: the qc0 score matmul for key chunk
                # sc consumes kt[sc] at slot ~2*sc+1, so every k-fill
                # must be emitted strictly before then (stale-K reads
                # otherwise corrupt exactly qc0)
                deferred = (by_kind[1][1] + [dma_units[2]]
                            + by_kind[2][1] + [dma_units[3]]
                            + by_kind[3][1] + by_kind[1][0])
                for sc in (1, 2, 3):
                    deferred.extend(by_kind[sc][2])
                    if sc < 3:
                        deferred.extend(by_kind[sc + 1][0])
                return boot_units, deferred

            # ---- attention tail units (per q-chunk qc) ---------------------
            def u_av(b, qc, qb, h):
                va = st[b]["va"]
                if h == 0:
                    ct = psc.tile([128, 2, 65], F32, tag="ct",
                                  name=f"ct{b}_{qc}_{qb}")
                    st[b][f"ct{qc}_{qb}"] = ct
                else:
                    ct = st[b][f"ct{qc}_{qb}"]
                for kb in range(NKB):
                    g, j = divmod(kb, 2)
                    at2 = st[b][f"at{qc}_{h}_{g}"]
                    nc.tensor.matmul(
                        ct[:, h, :],
                        at2[:, j, qb * 128:(qb + 1) * 128],
                        va[:, kb, h * 65:h * 65 + 65],
                        start=(kb == 0), stop=(kb == NKB - 1))

            def u_norm(b, qc, qb):
                ct = st[b][f"ct{qc}_{qb}"]
                rcp = zp.tile([128, 2, 1], F32, tag="rc",
                              name=f"rc{b}_{qc}_{qb}")
                with nc.allow_low_precision(reason="1/Z approx is fine"):
                    nc.vector.reciprocal(rcp[:], ct[:, :, 64:65])
                nctx = nxp.tile([128, 2, 64], F16, tag="nc",
                                name=f"nx{b}_{qc}_{qb}")
                st[b][f"nx{qc}_{qb}"] = nctx
                for h in range(2):
                    if drain_mode["on"]:
                        # post-exp drain: ACT is idle, DVE is the tail
                        # critical path
                        nc.scalar.activation(nctx[:, h, :], ct[:, h, 0:64],
                                             AF.Copy, scale=rcp[:, h, :])
                    else:
                        nc.vector.tensor_scalar_mul(nctx[:, h, :],
                                                    ct[:, h, 0:64],
                                                    rcp[:, h, :])

            def u_ctxT(b, qc, qb):
                nctx = st[b][f"nx{qc}_{qb}"]
                tp = pso.tile([128, F], F32, tag="o",
                              name=f"tc{b}_{qc}_{qb}")
                psoT["cnt"] += 1
                tpv = tp[:, 0:64].bitcast(F16)
                nc.tensor.transpose(
                    tpv, nctx[:].rearrange("p a b -> p (a b)"), identh[:])
                ctxT = cxp.tile([128, 128], F16, tag="cx",
                                name=f"cx{b}_{qc}_{qb}")
                st[b][f"cxT{qc}_{qb}"] = ctxT
                nc.vector.tensor_copy(ctxT[:], tpv)

            def u_oproj(b, qc, qb):
                # both 512-wide halves of the output row block: one
                # 4KB-per-row DMA instead of two (HWDGE is per-DMA cost)
                ctxT = st[b][f"cxT{qc}_{qb}"]
                ot = op.tile([128, D], F16, tag="ot",
                             name=f"ot{b}_{qc}_{qb}")
                last = drain_mode["on"]
                row = b * S + qc * F + qb * 128
                for jc in range(D // F):
                    ops = pso.tile([128, F], F32, tag="o",
                                   name=f"op{b}_{qc}_{qb}_{jc}")
                    psoT["cnt"] += 1
                    nc.tensor.matmul(ops[:], ctxT[:],
                                     wo_sb[:, jc * F:(jc + 1) * F],
                                     start=True, stop=True)
                    if last and (qb + jc) % 2 == 0:
                        # post-exp drain: ACT is idle once the final exp
                        # has issued — split the copies between ACT and DVE
                        nc.scalar.activation(ot[:, jc * F:(jc + 1) * F],
                                             ops[:], AF.Copy)
                    else:
                        nc.vector.tensor_copy(ot[:, jc * F:(jc + 1) * F],
                                              ops[:])
                nc.sync.dma_start(out_d[row:row + 128, :], ot[:])

            def tail_units(b, qc):
                mk = lambda f, *a: (lambda: f(b, qc, *a))
                av = [(432, mk(u_av, qb, h), "")
                      for qb in range(4) for h in range(2)]
                nm = [(0, mk(u_norm, qb), "") for qb in range(4)]
                ctx = [(53, mk(u_ctxT, qb), "pso") for qb in range(4)]
                # AV units free the at2 buffers the next q-chunk's exps
                # reuse — they MUST land early in the next chunk's PE
                # stream (av_q has guaranteed per-slot progress), else the
                # in-order PE/ACT queues deadlock. Same for norm/transpose
                # units whose nctx buffers rotate. Units are kept under
                # ~450ns so a slot's drains don't stretch the score-matmul
                # cadence past the ACT exp period. Chains run depth-first
                # so each q-block's out-proj becomes runnable early.
                return [av[0], av[1], av[2], av[3], nm[0], ctx[0],
                        av[4], av[5], nm[1], ctx[1],
                        av[6], av[7], nm[2], ctx[2], nm[3], ctx[3]]

            def proj_units(b, qc):
                return [(b, qc, qb, (lambda qb=qb: u_oproj(b, qc, qb)))
                        for qb in range(4)]

            # ---- emission --------------------------------------------------
            boot, deferred_boot = a_units(0, defer_boot=True)
            # PE p-state warmup: the tensor engine runs at half clock for
            # its first 3us of continuous work. Burn the ramp on dummy
            # matmuls (no DMA deps) so the real fills run at full speed
            # the moment the x/w DMAs land.
            jk = wp.tile([128, F], F16, tag="jk")
            nc.vector.memset(jk[:], 0.0)
            for w in range(8):
                wps = psb.tile([128, 2, F], F32, tag="sc", name=f"warm{w}")
                nc.tensor.matmul(wps[:, w % 2, :], jk[:, 0:128], jk[:],
                                 start=True, stop=True)
            # boot DMA order: critical path to the first k/q fill chains
            # (wq, xt chunk 0, biases, wk, wv), then everything the first
            # q-chunk's tail needs (identh/wo after chunk 1)
            nc.sync.dma_start(
                wq_sb[:].rearrange("p kt hl m -> p (kt hl m)"), wq_d[:])
            boot[1][1]()       # first xt chunk
            load_fill_weights()
            boot[0][1]()       # tile allocs + ones memsets
            # preload the Exp activation table while the PE does batch-0
            # QKV; scale=0 makes the input values irrelevant (exp(0)=1)
            junk = zp.tile([1, 32], F32, tag="junk")
            nc.scalar.activation(junk[:], jk[0:1, 0:64].bitcast(F32),
                                 AF.Exp, scale=0.0)
            for u in boot[2:]:
                u[1]()
            load_const()
            a_q = []
            av_q = list(deferred_boot)  # deadline-critical units
            tail_q = []         # (unused spillover queue)
            p_q = []            # deferred out-proj units (lowest priority)
            P_PE = 426          # PE-ns of one out-proj unit (2 matmuls)
            SLOT_PE = 900       # slack-fill target per slot pair (x2)
            SLOT_CAP = 1010     # last batch: hard per-pair PE cap (x2)
            NPOP_TH = 16        # av_q backlog above which 4 units/pair pop
            FLOOR = 0           # out-proj backlog reserve (spend greedily)

            def run_u(u):
                u[1]()
                return u[0]

            def pop_allowed(q):
                # FIFO pop skipping units whose ps_o allocation would land
                # on a bank still owned by an open fill chain. Once a
                # chain-opening unit is skipped, its mid/close units must
                # not be popped ahead of it.
                blocked_chain = False
                for i, u in enumerate(q):
                    k = u[2]
                    if blocked_chain and k in ("mid", "close"):
                        continue
                    if pso_ok(k):
                        return q.pop(i)
                    if k == "open":
                        blocked_chain = True
                return None

            def pop_p():
                # out-proj units are only runnable once their transposed
                # context tile exists (and never while a fill chain holds a
                # ps_o bank: u_oproj allocates two); skip-scan in FIFO order
                if psoT["live"]:
                    return False
                for i, (pb, pqc, pqb, fn) in enumerate(p_q):
                    if f"cxT{pqc}_{pqb}" in st[pb]:
                        p_q.pop(i)
                        fn()
                        return True
                return False

            for b in range(B):
                if b + 1 < B:
                    a_q.extend(a_units(b + 1))
                qt, kt = st[b]["qt"], st[b]["kt"]
                import os
                if os.environ.get("KERNEL_DEBUG_QUEUES"):
                    print(f"batch {b}: av={len(av_q)} a={len(a_q)} "
                          f"p={len(p_q)}")
                # batch-level quota in PE-ns: spread current queues plus
                # the tail units arriving from qc 0..2 over the 64 slots
                pend_pe = (sum(u[0] for u in av_q)
                           + sum(u[0] for u in tail_q)
                           + sum(u[0] for u in a_q) + 3 * 3669)
                if b == B - 1:
                    pend_pe += P_PE * (len(p_q) + 16)
                done_pe = 0
                gi = 0
                for qc in range(NSC):
                    qlo = qc * F
                    if os.environ.get("KERNEL_DEBUG_QUEUES"):
                        print(f"  b{b} qc{qc}: av={len(av_q)} "
                              f"a={len(a_q)} p={len(p_q)} done={done_pe} "
                              f"pend={pend_pe}")
                    # out-proj reserve: batches 0-2 bank a backlog for the
                    # ACT-bound last batch (sized to its DVE copy
                    # capacity), which spends it evenly across its chunks
                    floor = FLOOR
                    for g in range(NKB // 2):
                        for h in range(2):
                            hp = h * 64
                            sc2 = psb.tile([128, 2, F], F32, tag="sc",
                                           name=f"s{b}{qc}{h}{g}")
                            for j in range(2):
                                kb = g * 2 + j
                                nc.tensor.matmul(
                                    sc2[:, j, :],
                                    kt[hp:hp + 64, kb * 128:(kb + 1) * 128],
                                    qt[hp:hp + 64, qlo:qlo + F],
                                    start=True, stop=True)
                            at2 = apool.tile([128, 2, F], F16, tag="at",
                                             name=f"a{b}{qc}{h}{g}")
                            st[b][f"at{qc}_{h}_{g}"] = at2
                            nc.scalar.activation(at2[:], sc2[:], AF.Exp,
                                                 scale=EXP_SCALE)
                            # paired slots: the 2-buffer score pipeline
                            # lets two groups issue back-to-back; draining
                            # once per pair doubles the cadence tolerance
                            gi += 1
                            if h == 0:
                                continue
                            slot_pe = 2 * 426
                            # guaranteed progress for deadline-critical
                            # AV units (at2 buffer recycling); drain double
                            # when backlogged (batch-0 deferred boot)
                            npop = 4 if len(av_q) > NPOP_TH else 2
                            for _ in range(min(npop, len(av_q))):
                                u = pop_allowed(av_q)
                                if u is None:
                                    break
                                pe = run_u(u)
                                done_pe += max(pe, 60)
                                slot_pe += pe
                            quota = (gi * pend_pe) // 64
                            while done_pe < quota and (av_q or tail_q
                                                       or a_q
                                                       or len(p_q) > floor):
                                if b == B - 1 and slot_pe >= 2 * SLOT_CAP:
                                    break
                                ape = sum(u[0] for u in a_q)
                                tpe = (sum(u[0] for u in av_q)
                                       + sum(u[0] for u in tail_q))
                                u = None
                                if (av_q or tail_q) and (tpe >= ape
                                                         or not a_q):
                                    u = pop_allowed(av_q if av_q
                                                    else tail_q)
                                if u is None and a_q and pso_ok(a_q[0][2]):
                                    u = a_q.pop(0)
                                if u is not None:
                                    pe = run_u(u)
                                else:
                                    if not pop_p():
                                        break
                                    pe = P_PE
                                done_pe += max(pe, 60)
                                slot_pe += pe
                            # fill any remaining slot slack with deferred
                            # out-proj work; retain a backlog to fill the
                            # ACT-bound last batch (fill pops do not
                            # count toward the batch quota)
                            cap = 2 * (SLOT_PE if b < B - 1
                                       else SLOT_CAP - P_PE)
                            while (slot_pe < cap and len(p_q) > floor
                                   and pop_p()):
                                slot_pe += P_PE
                    av_q.extend(tail_units(b, qc))
                    p_q.extend(proj_units(b, qc))
            drain_mode["on"] = True
            while av_q or tail_q or a_q or p_q:
                u = pop_allowed(av_q) or pop_allowed(tail_q)
                if u is not None:
                    run_u(u)
                elif a_q and pso_ok(a_q[0][2]):
                    run_u(a_q.pop(0))
                elif not pop_p():
                    raise RuntimeError("unrunnable units left")
                # start ready out-proj chains immediately so their DVE
                # copies + DMAs overlap the remaining tail
                pop_p()

    nc.compile()
    return nc


def _fp8_split(a):
    # hi + lo fp8 decomposition: a ~ hi + lo with ~3e-3 relative error
    hi = a.astype(E4M3)
    lo = (a - hi.astype(np.float32)).astype(E4M3)
    return hi, lo


def _shuf8(w):
    # [D, DPC] f32 -> [128, KT, 2, DPC] fp8 so w8[p, kt, hl, m] matches
    # w_hl[kt*128+p, m]
    hi, lo = _fp8_split(w)
    out = np.empty((128, KT, 2, w.shape[1]), E4M3)
    out[:, :, 0] = hi.reshape(KT, 128, -1).transpose(1, 0, 2)
    out[:, :, 1] = lo.reshape(KT, 128, -1).transpose(1, 0, 2)
    return np.ascontiguousarray(out.reshape(128, -1))


def _host_inputs(x, Wq, bq, Wk, bk, Wv, bv, Wo, bo):
    x2 = np.ascontiguousarray(np.asarray(x, np.float32).reshape(SEQ, D))
    xT = np.ascontiguousarray(x2.T)
    xhi, xlo = _fp8_split(xT)
    xT8 = np.empty((128, 2, KT, SEQ), E4M3)
    xT8[:, 0] = xhi.reshape(KT, 128, SEQ).transpose(1, 0, 2)
    xT8[:, 1] = xlo.reshape(KT, 128, SEQ).transpose(1, 0, 2)
    xT8 = np.ascontiguousarray(xT8.reshape(128, -1))
    identh = np.eye(128, dtype=np.float16)
    in_maps = []
    for c in range(NCORES):
        sl = slice(c * DPC, (c + 1) * DPC)
        woT = np.ascontiguousarray(
            np.asarray(Wo, np.float32)[:, sl].T / WSC).astype(np.float16)
        in_maps.append({
            "xT8": xT8,
            "wq8": _shuf8(np.asarray(Wq, np.float32)[sl].T * WSC),
            "wk8": _shuf8(np.asarray(Wk, np.float32)[sl].T * WSC),
            "wv8": _shuf8(np.asarray(Wv, np.float32)[sl].T * WSC),
            "constT": np.concatenate([identh, woT], axis=1),
            "bqk": np.stack([np.asarray(bq, np.float32)[sl] * WSC,
                             np.asarray(bk, np.float32)[sl] * WSC],
                            axis=1),
        })
    return in_maps


def _run(inputs, trace=False, trace_kwargs=None):
    if "nc" not in _CACHE:
        _CACHE["nc"] = _build()
    nc = _CACHE["nc"]
    in_maps = _host_inputs(**inputs)
    res = run_bass_kernel_spmd(nc, in_maps, list(range(NCORES)), trace=trace,
                               **(trace_kwargs or {}))
    acc = res.results[0]["out"].astype(np.float32).copy()
    for c in range(1, NCORES):
        acc += res.results[c]["out"]
    acc += np.asarray(inputs["bo"], np.float32)[None, :]
    # bv is folded here instead of on-device: attention weights sum to 1,
    # so the V-bias contributes exactly bv @ Wo^T to every output row
    acc += (np.asarray(inputs["bv"], np.float32)
            @ np.asarray(inputs["Wo"], np.float32).T)[None, :]
    return acc.reshape(B, S, D), res


def kernel(**inputs):
    out, _ = _run(inputs)
    return out
